# revision 8
# baseline (speedup 1.0000x reference)
"""TRN2 Bass kernel for nn_AttentionMP (GNN message passing attention).

Row-parallel attention across 8 NeuronCores: core c owns query rows
[c*1024, (c+1)*1024). Scores are computed TRANSPOSED, sT[j, i] (j = key
index on partitions, i = this core's query rows on the free dim), which
makes att^T directly available as the moving operand of downstream
matmuls — no on-device transposes in the hot path. The tiny
data-independent products fold on the host: qk = (Wq Wk^T)^T Hq^T ships
per-core (so no on-device q/k projection at all), W1v = Wv @ W1 ships
replicated (eliminating the v projection together with the Z
reassociation below).

Masking is split across engines to balance the pipeline (the PE would
otherwise be the bottleneck at 3 matmul-passes/tile):
 - j-tiles with (jt % 8) in {1,4,7} (includes tile 63): additive mask on
   the PE — adj ships as fp8 and lands in the scores PSUM as 240*adj via
   an identity matmul (lhsT = 240*I fp8); ACT computes exp(s + 240*m - 270):
   exp(s-30) unmasked, 0.0 exactly for masked entries.
 - the rest (5/8, includes tile 0 so the first exp needs no mask matmul
   and tile 63 never needs the DVE mult): multiplicative mask on the DVE
   — scores skip the
   mask matmul, ACT computes exp(s - 30) unmasked (bf16, finite), then
   e *= adj with adj shipped as bf16 (0/1 exact, all-bf16 tensor_mult
   runs in 2x_1p mode).
The -30 is a global stabilizer that cancels in normalization.

e is bf16: the softmax accumulator runs on DVE in bf16 (2x_1p) and
att@v is reassociated into Z[c,i] = sum_j H[j,c] e[j,i], accumulated in
PSUM across j-tiles with bf16 H as lhsT (natural-layout pretiled HN
chunks). PSUM: one triple-buffered pool of [128,1024] tiles (6 banks) +
the Z accumulator (2 banks); small stage-2 outputs use slices of the big
pool tiles. Because relu commutes with positive per-row scaling, softmax
normalization is deferred through the whole MLP:
    out = relu(relu(U@W1 + d*b1)@W2 + d*b2) / d,   U = Z^T @ Wv
so the MLP runs transposed with stationary weights and d*b enters via
rank-1 matmuls (den = ones @ (acc + e_last): the last tile's e never
touches the DVE accumulator — it rides a second accumulating matmul).
The final layer is re-flipped: per 128-row block, lhsT = hidden block
(SBUF) x W2 puts the output non-transposed in PSUM, and relu with the
per-row 1/d scale (ACT/DVE alternating) writes the staging tile
directly — no transposes and no extra PSUM->SBUF copy in the tail.
"""
import numpy as np
import ml_dtypes
import concourse.bass as bass
from concourse import bacc
import concourse.mybir as mybir
from concourse.tile import TileContext
from concourse.bass_utils import run_bass_kernel_spmd

N = 8192
D = 128
NC = 8
RPC = N // NC          # rows per core = 1024
JT = N // 128          # j tiles = 64
F32 = mybir.dt.float32
F32R = mybir.dt.float32r
BF16 = mybir.dt.bfloat16
FP8 = mybir.dt.float8e4
MASK_D = 240.0         # fp8e4 max finite
STAB = 30.0            # global score shift, cancels in softmax
HT_CHUNKS = 4
GRP = 8                # j-tiles per adj DMA batch group
N_WARM = 26            # HAM-warmup matmuls at kernel start

PE_MASKED = ([5, 7] + [8 + r for r in (1, 3, 5, 7)]
             + [8 * g + r for g in range(2, 8) for r in (1, 4, 7)])
DVE_MASKED = [jt for jt in range(JT) if jt not in PE_MASKED]
NE = len(PE_MASKED)    # 24
NO = len(DVE_MASKED)   # 40
E_IDX = {jt: i for i, jt in enumerate(PE_MASKED)}
O_IDX = {jt: i for i, jt in enumerate(DVE_MASKED)}
E_CNT = [sum(1 for jt in PE_MASKED if jt // 8 == g) for g in range(8)]
O_CNT = [sum(1 for jt in DVE_MASKED if jt // 8 == g) for g in range(8)]
E_BASE = [sum(E_CNT[:g]) for g in range(8)]
O_BASE = [sum(O_CNT[:g]) for g in range(8)]

_CACHED = {}


def build(with_bias=False):
    nc = bacc.Bacc("TRN2", target_bir_lowering=False, debug=True)

    HTC = [nc.dram_tensor(f"HT{t}", [D, N // HT_CHUNKS], F32R, kind="ExternalInput")
           for t in range(HT_CHUNKS)]
    HNC = [nc.dram_tensor(f"HN{t}", [D, N // 4], BF16, kind="ExternalInput")
           for t in range(4)]  # pretiled [p, t*128+c], bf16
    QK = nc.dram_tensor("QK", [D, RPC], F32R, kind="ExternalInput")
    ADJE = nc.dram_tensor("ADJE", [NE * 128, RPC], FP8, kind="ExternalInput")
    ADJO = nc.dram_tensor("ADJO", [NO * 128, RPC], BF16, kind="ExternalInput")
    W1V = nc.dram_tensor("W1V", [D, D], F32R, kind="ExternalInput")
    W2 = nc.dram_tensor("W2", [D, D], BF16, kind="ExternalInput")
    B1R = nc.dram_tensor("B1R", [1, D], F32R, kind="ExternalInput")
    B2R = nc.dram_tensor("B2R", [1, D], F32R, kind="ExternalInput")
    I240 = nc.dram_tensor("I240", [D, D], FP8, kind="ExternalInput")
    ONES = nc.dram_tensor("ONES", [D, D], BF16, kind="ExternalInput")
    IDENT = nc.dram_tensor("IDENT", [1, 1], F32, kind="ExternalInput")
    BIASE = nc.dram_tensor("BIASE", [D, 1], F32, kind="ExternalInput")
    BIASO = nc.dram_tensor("BIASO", [D, 1], F32, kind="ExternalInput")
    OUT = nc.dram_tensor("OUT", [RPC, D], F32, kind="ExternalOutput")

    adjE_rows = ADJE.rearrange("(t p) i -> p t i", p=128)  # [p, E-tile, i]
    adjO_rows = ADJO.rearrange("(t p) i -> p t i", p=128)  # [p, O-tile, i]

    # group g covers j-tiles [g*8, g*8+8): 3 PE-masked + 5 DVE-masked
    NG = JT // GRP

    with TileContext(nc) as tc:
        with (
            tc.tile_pool(name="pers", bufs=1) as pers,
            tc.tile_pool(name="adjE", bufs=4) as adjEp,
            tc.tile_pool(name="adjO", bufs=4) as adjOp,
            tc.tile_pool(name="ep", bufs=8) as ep,
            tc.tile_pool(name="psA", bufs=3, space="PSUM") as psA,   # 3x[128,1024]
            tc.tile_pool(name="psZ", bufs=1, space="PSUM") as psZ,   # Z accumulator
        ):
            # ---- persistent tiles
            htc = []
            for t in range(HT_CHUNKS):
                htc_t = pers.tile([D, N // HT_CHUNKS], F32R, tag=f"ht{t}")
                htc.append(htc_t)
            hnc = []
            for t in range(4):
                hnc_t = pers.tile([D, N // 4], BF16, tag=f"hn{t}")
                hnc.append(hnc_t)
            qk = pers.tile([D, RPC], F32R, tag="qk")
            w1v = pers.tile([D, D], F32R, tag="w1v")
            w2 = pers.tile([D, D], BF16, tag="w2")
            b1r = pers.tile([1, D], F32R, tag="b1r")
            b2r = pers.tile([1, D], F32R, tag="b2r")
            i240 = pers.tile([D, D], FP8, tag="i240")
            ones = pers.tile([D, D], BF16, tag="ones")
            ident = pers.tile([1, 1], F32, tag="ident")
            biase = pers.tile([D, 1], F32, tag="biase")
            biaso = pers.tile([D, 1], F32, tag="biaso")

            def e_group_tile(g):
                return adjEp.tile([128, 4 * RPC], FP8, tag="ae", name=f"ae{g}")

            def o_group_tile(g):
                return adjOp.tile([128, 6 * RPC], BF16, tag="ao", name=f"ao{g}")

            def e_base(g):  # first ADJE tile index of group g
                return E_BASE[g]

            def o_base(g):
                return O_BASE[g]

            # critical-path DMAs on the sync queue, most-urgent first
            # (packets drain roughly in issue order); bulk/late tensors go
            # on gpsimd's software queue.
            aE0 = e_group_tile(0)
            aO0 = o_group_tile(0)
            aE1 = e_group_tile(1)
            aO1 = o_group_tile(1)
            aE = {0: aE0, 1: aE1}
            aO = {0: aO0, 1: aO1}

            hchalf0 = N // HT_CHUNKS // 2

            def adj_tile_dma(jt):
                """Issue the DMA for j-tile jt's adj slice (per-tile grain)."""
                g = jt // GRP
                if jt in E_IDX:
                    half = E_IDX[jt] - e_base(g)
                    nc.sync.dma_start(out=aE[g][:, half * RPC:(half + 1) * RPC],
                                      in_=adjE_rows[:, E_IDX[jt]])
                else:
                    half = O_IDX[jt] - o_base(g)
                    nc.sync.dma_start(out=aO[g][:, half * RPC:(half + 1) * RPC],
                                      in_=adjO_rows[:, O_IDX[jt]])

            nc.sync.dma_start(out=qk[:, 0:512], in_=QK[:, 0:512])
            nc.sync.dma_start(out=biaso[:], in_=BIASO[:])
            nc.sync.dma_start(out=htc[0][:, 0:256], in_=HTC[0][:, 0:256])
            nc.sync.dma_start(out=qk[:, 512:1024], in_=QK[:, 512:1024])
            adj_tile_dma(0)
            nc.sync.dma_start(out=htc[0][:, 256:768], in_=HTC[0][:, 256:768])
            # first Z weights on the fast queue: hnc0's head columns feed
            # Z(0..3); the slow gpsimd software queue ships the rest.
            nc.sync.dma_start(out=hnc[0][:, 0:512], in_=HNC[0][:, 0:512])
            nc.sync.dma_start(out=i240[:], in_=I240[:])
            adj_tile_dma(1)
            nc.sync.dma_start(out=aE0[:, 0:E_CNT[0] * RPC].rearrange(
                "p (k i) -> p k i", k=E_CNT[0]),
                in_=adjE_rows[:, 0:E_CNT[0]])
            nc.sync.dma_start(out=biase[:], in_=BIASE[:])
            adj_tile_dma(2)
            nc.sync.dma_start(out=htc[0][:, 768:N // HT_CHUNKS],
                              in_=HTC[0][:, 768:N // HT_CHUNKS])
            adj_tile_dma(3)
            adj_tile_dma(4)
            adj_tile_dma(6)
            for gg in (1, 2):
                if gg == 2:
                    aE[2] = e_group_tile(2)
                    aO[2] = o_group_tile(2)
                    nc.sync.dma_start(out=htc[1][:, 0:hchalf0],
                                      in_=HTC[1][:, 0:hchalf0])
                nc.sync.dma_start(
                    out=aE[gg][:, 0:E_CNT[gg] * RPC].rearrange(
                        "p (k i) -> p k i", k=E_CNT[gg]),
                    in_=adjE_rows[:, E_BASE[gg]:E_BASE[gg] + E_CNT[gg]])
                nc.sync.dma_start(
                    out=aO[gg][:, 0:O_CNT[gg] * RPC].rearrange(
                        "p (k i) -> p k i", k=O_CNT[gg]),
                    in_=adjO_rows[:, O_BASE[gg]:O_BASE[gg] + O_CNT[gg]])
                if gg == 1:
                    nc.sync.dma_start(out=hnc[0][:, 512:], in_=HNC[0][:, 512:])
                else:
                    nc.sync.dma_start(out=htc[1][:, hchalf0:],
                                      in_=HTC[1][:, hchalf0:])
            for t, src in [(ident, IDENT), (w1v, W1V), (w2, W2),
                           (b1r, B1R), (b2r, B2R), (ones, ONES)]:
                nc.gpsimd.dma_start(out=t[:], in_=src[:])
            nc.gpsimd.dma_start(out=hnc[1][:], in_=HNC[1][:])

            acc = pers.tile([D, RPC], BF16, tag="acc")
            zsb = pers.tile([D, RPC], F32R, tag="zsb")
            hts = pers.tile([D, RPC], BF16, tag="hts")    # hidden^T (SBUF, bf16)
            dentr = pers.tile([1, RPC], F32R, tag="dentr")
            rcol = pers.tile([D, NC], F32, tag="rcol")
            outsb = pers.tile([D, NC * D], F32, tag="outsb")

            # ---- stage 1 (Z matmuls lag two j-tiles so scores(jt+1)
            # issue while exp(jt) runs)
            zps = psZ.tile([D, RPC], F32, tag="z")

            # HAM warmup: the PE sits DMA-gated ~13us at kernel start and the
            # clock gate (K=4/8, 1.2GHz) only releases after ~3.4us of
            # sustained matmul activity. Without this the first ~80 real
            # matmuls run at half clock (~18us lost). Run dependency-free
            # matmuls on memset zeros into the psZ banks (the first real Z
            # matmul has start=True, so the garbage is cleared) while DMAs
            # stream in. The dummy exp pulls ACT_TABLE_LOAD off the critical
            # path too.
            warm = pers.tile([D, 512], BF16, tag="warm")
            nc.vector.memset(warm[:], 0.0)
            nc.scalar.activation(warm[:, 0:1], warm[:, 0:1],
                                 mybir.ActivationFunctionType.Exp)
            for w in range(N_WARM):
                nc.tensor.matmul(zps[:, 0:512], lhsT=warm[:, 0:128],
                                 rhs=warm[:], start=True, stop=True)

            etiles = {}

            def do_z(jt):
                e_prev = etiles.pop(jt)
                htile = hnc[jt // 16][:, (jt % 16) * 128:(jt % 16 + 1) * 128]
                for h in range(2):
                    cs = slice(h * 512, (h + 1) * 512)
                    nc.tensor.matmul(zps[:, cs], lhsT=htile, rhs=e_prev[:, cs],
                                     start=(jt == 0), stop=(jt == JT - 1))

            cwq = N // HT_CHUNKS // 128

            def mask_mms(jt, sps_t):
                """Additive-mask matmuls for PE-masked tile jt (start=True).

                Issued one tile AHEAD of jt's score matmuls: an accumulating
                matmul (start=False) can't start streaming until its
                predecessor on the same PSUM region fully drains (~420ns
                in-pair spacing vs 216ns normal). With the mask pair issued
                early, the intervening score/Z matmuls cover the drain."""
                g = jt // GRP
                half = E_IDX[jt] - e_base(g)
                adj_sb = aE[g]
                for h in range(2):
                    cs = slice(h * 512, (h + 1) * 512)
                    nc.tensor.matmul(sps_t[:, cs], lhsT=i240[:],
                                     rhs=adj_sb[:, half * RPC + h * 512:
                                                half * RPC + (h + 1) * 512],
                                     start=True, stop=False)

            sps_next = psA.tile([D, RPC], F32, tag="big")
            if 0 in E_IDX:
                mask_mms(0, sps_next)
            for jt in range(JT):
                g, r = divmod(jt, GRP)
                # per-tile prefetch 2 groups ahead keeps the wires smooth:
                # one adj tile per iteration, H chunks in halves mid-group.
                tgt = jt + 3 * GRP
                if tgt < JT:
                    if tgt % GRP == 0:
                        gg = tgt // GRP
                        aE[gg] = e_group_tile(gg)
                        aO[gg] = o_group_tile(gg)
                    adj_tile_dma(tgt)
                hchalf = N // HT_CHUNKS // 2
                for gc in (2, 3):
                    if jt == 16 * gc - 14:
                        nc.sync.dma_start(out=htc[gc][:, 0:hchalf],
                                          in_=HTC[gc][:, 0:hchalf])
                    elif jt == 16 * gc - 10:
                        nc.sync.dma_start(out=htc[gc][:, hchalf:],
                                          in_=HTC[gc][:, hchalf:])
                for gc in (2, 3):
                    if jt == 16 * gc - 6:
                        nc.sync.dma_start(out=hnc[gc][:], in_=HNC[gc][:])
                even = jt in E_IDX
                sps = sps_next
                if jt + 1 < JT:
                    sps_next = psA.tile([D, RPC], F32, tag="big")
                    if (jt + 1) in E_IDX:
                        mask_mms(jt + 1, sps_next)
                ktile = htc[jt // cwq][:, (jt % cwq) * 128:(jt % cwq + 1) * 128]
                for h in range(2):
                    cs = slice(h * 512, (h + 1) * 512)
                    nc.tensor.matmul(sps[:, cs], lhsT=ktile, rhs=qk[:, cs],
                                     start=not even, stop=True)
                e = ep.tile([D, RPC], BF16, tag="e")
                nc.scalar.activation(e[:], sps[:],
                                     mybir.ActivationFunctionType.Exp,
                                     bias=(biase[:] if even else biaso[:]))
                etiles[jt] = e
                if not even:
                    half = O_IDX[jt] - o_base(g)
                    adjo_sb = aO[g]
                    nc.vector.tensor_mul(e[:], e[:],
                                         adjo_sb[:, half * RPC:(half + 1) * RPC])
                if jt == 0:
                    nc.vector.tensor_copy(acc[:], e[:])
                elif jt < JT - 1:
                    nc.vector.tensor_add(acc[:], acc[:], e[:])
                if jt >= 2:
                    do_z(jt - 2)
            do_z(JT - 2)
            e_last = etiles[JT - 1]

            # ---- stage 2: denominators + normalization-deferred transposed MLP
            # z63 first: it gates the zsb -> gps -> hts -> blocks chain,
            # while the den matmuls only feed the (slack-rich) rcol path.
            do_z(JT - 1)
            # den = ones @ (acc + e_last): e(63)'s DVE add is skipped — it
            # rides a second accumulating matmul instead.
            dps = psA.tile([D, RPC], F32, tag="big")
            for h in range(2):
                cs = slice(h * 512, (h + 1) * 512)
                nc.tensor.matmul(dps[:, cs], lhsT=ones[:], rhs=acc[:, cs],
                                 start=True, stop=False)
            for h in range(2):
                cs = slice(h * 512, (h + 1) * 512)
                nc.tensor.matmul(dps[:, cs], lhsT=ones[:], rhs=e_last[:, cs],
                                 start=False, stop=True)
            nc.vector.tensor_copy(dentr[:, 0:512], dps[0:1, 0:512])
            nc.scalar.copy(zsb[:, 0:512], zps[:, 0:512])
            nc.vector.tensor_copy(dentr[:, 512:1024], dps[0:1, 512:1024])
            nc.vector.tensor_copy(zsb[:, 512:1024], zps[:, 512:1024])
            gps = psA.tile([D, RPC], F32, tag="big")
            for h in range(2):
                cs = slice(h * 512, (h + 1) * 512)
                nc.tensor.matmul(gps[:, cs], lhsT=w1v[:], rhs=zsb[:, cs],
                                 start=True, stop=not with_bias)
                if with_bias:
                    nc.tensor.matmul(gps[:, cs], lhsT=b1r[:], rhs=dentr[:, cs],
                                     start=False, stop=True)
            # 1/denom columns via tiny PE transposes of the den row
            rps = psA.tile([D, RPC], F32, tag="big")
            for it in range(NC):
                nc.tensor.transpose(rps[:, it:it + 1],
                                    dentr[0:1, it * 128:(it + 1) * 128].bitcast(F32),
                                    ident[0:1, 0:1])
            nc.scalar.activation(hts[:, 0:512], gps[:, 0:512],
                                 mybir.ActivationFunctionType.Relu)
            nc.vector.tensor_relu(hts[:, 512:1024], gps[:, 512:1024])
            nc.vector.reciprocal(rcol[:], rps[:, 0:NC])
            # final layer, re-flipped per 128-row block: lhsT = hts block so
            # the output block lands non-transposed in PSUM; relu + (1/d)
            # scale alternates ACT/DVE and writes the staging tile directly.
            outv = OUT.rearrange("(t p) d -> p t d", p=128)
            for it in range(NC):
                bps = psA.tile([D, RPC], F32, tag="big")
                nc.tensor.matmul(bps[:, 0:D], lhsT=hts[:, it * 128:(it + 1) * 128],
                                 rhs=w2[:], start=True, stop=not with_bias)
                if with_bias:
                    nc.tensor.matmul(bps[:, 0:D],
                                     lhsT=dentr[0:1, it * 128:(it + 1) * 128],
                                     rhs=b2r[:], start=False, stop=True)
                ob = outsb[:, it * 128:(it + 1) * 128]
                if it % 2 == 0:
                    nc.scalar.activation(ob, bps[:, 0:D],
                                         mybir.ActivationFunctionType.Relu,
                                         scale=rcol[:, it:it + 1])
                else:
                    nc.vector.tensor_scalar(ob, bps[:, 0:D], rcol[:, it:it + 1],
                                            0.0, op0=mybir.AluOpType.mult,
                                            op1=mybir.AluOpType.max)
                if it % 2 == 1:
                    nc.sync.dma_start(
                        out=outv[:, it - 1:it + 1],
                        in_=outsb[:, (it - 1) * D:(it + 1) * D].rearrange(
                            "p (t d) -> p t d", t=2))
    nc.finalize()
    return nc


def _prep(H, adj, Wq, Wk, Wv, W1, b1, W2, b2):
    f8 = ml_dtypes.float8_e4m3
    bf = ml_dtypes.bfloat16
    H32 = np.asarray(H, dtype=np.float32)
    HT = np.ascontiguousarray(H32.T)
    adj = np.asarray(adj)
    M = (np.asarray(Wq, np.float32) @ np.asarray(Wk, np.float32).T)
    base = {
        "W1V": np.ascontiguousarray(np.asarray(Wv, np.float32) @ np.asarray(W1, np.float32)),
        "W2": np.asarray(W2, np.float32).astype(bf),
        "B1R": np.asarray(b1, np.float32).reshape(1, D),
        "B2R": np.asarray(b2, np.float32).reshape(1, D),
        "I240": (np.eye(D, dtype=np.float32) * MASK_D).astype(f8),
        "ONES": np.ones((D, D), bf),
        "IDENT": np.eye(1, dtype=np.float32),
        "BIASE": np.full((D, 1), -(MASK_D + STAB), np.float32),
        "BIASO": np.full((D, 1), -STAB, np.float32),
    }
    cw = N // HT_CHUNKS
    for t in range(HT_CHUNKS):
        base[f"HT{t}"] = np.ascontiguousarray(HT[:, t * cw:(t + 1) * cw])
    HNP = np.ascontiguousarray(
        H32.reshape(JT, 128, D).transpose(1, 0, 2).reshape(128, N)).astype(bf)
    for t in range(4):
        base[f"HN{t}"] = np.ascontiguousarray(HNP[:, t * (N // 4):(t + 1) * (N // 4)])
    in_maps = []
    for c in range(NC):
        m = dict(base)
        m["QK"] = np.ascontiguousarray(M.T @ HT[:, c * RPC:(c + 1) * RPC])
        adjT4 = np.ascontiguousarray(
            adj[c * RPC:(c + 1) * RPC, :].T).reshape(JT, 128, RPC)
        m["ADJE"] = np.ascontiguousarray(
            adjT4[PE_MASKED].reshape(NE * 128, RPC)).astype(np.float32).astype(f8)
        m["ADJO"] = np.ascontiguousarray(
            adjT4[DVE_MASKED].reshape(NO * 128, RPC)).astype(np.float32).astype(bf)
        in_maps.append(m)
    return in_maps


def kernel(H, adj, Wq, Wk, Wv, W1, b1, W2, b2):
    wb = bool(np.any(np.asarray(b1)) or np.any(np.asarray(b2)))
    key = f"nc{int(wb)}"
    if key not in _CACHED:
        _CACHED[key] = build(with_bias=wb)
    in_maps = _prep(H, adj, Wq, Wk, Wv, W1, b1, W2, b2)
    res = run_bass_kernel_spmd(_CACHED[key], in_maps, list(range(NC)))
    return np.concatenate([res.results[c]["OUT"] for c in range(NC)], axis=0)



# revision 10
# speedup vs baseline: 1.0135x; 1.0135x over previous
"""TRN2 Bass kernel for nn_AttentionMP (GNN message passing attention).

Row-parallel attention across 8 NeuronCores: core c owns query rows
[c*1024, (c+1)*1024). Scores are computed TRANSPOSED, sT[j, i] (j = key
index on partitions, i = this core's query rows on the free dim), which
makes att^T directly available as the moving operand of downstream
matmuls — no on-device transposes in the hot path. The tiny
data-independent products fold on the host: qk = (Wq Wk^T)^T Hq^T ships
per-core (so no on-device q/k projection at all), W1v = Wv @ W1 ships
replicated (eliminating the v projection together with the Z
reassociation below).

Masking is split across engines to balance the pipeline (the PE would
otherwise be the bottleneck at 3 matmul-passes/tile):
 - j-tiles with (jt % 8) in {1,4,7} (includes tile 63): additive mask on
   the PE — adj ships as fp8 and lands in the scores PSUM as 240*adj via
   an identity matmul (lhsT = 240*I fp8); ACT computes exp(s + 240*m - 270):
   exp(s-30) unmasked, 0.0 exactly for masked entries.
 - the rest (5/8, includes tile 0 so the first exp needs no mask matmul
   and tile 63 never needs the DVE mult): multiplicative mask on the DVE
   — scores skip the
   mask matmul, ACT computes exp(s - 30) unmasked (bf16, finite), then
   e *= adj with adj shipped as bf16 (0/1 exact, all-bf16 tensor_mult
   runs in 2x_1p mode).
The -30 is a global stabilizer that cancels in normalization.

e is bf16: the softmax accumulator runs on DVE in bf16 (2x_1p) and
att@v is reassociated into Z[c,i] = sum_j H[j,c] e[j,i], accumulated in
PSUM across j-tiles with bf16 H as lhsT (natural-layout pretiled HN
chunks). PSUM: one triple-buffered pool of [128,1024] tiles (6 banks) +
the Z accumulator (2 banks); small stage-2 outputs use slices of the big
pool tiles. Because relu commutes with positive per-row scaling, softmax
normalization is deferred through the whole MLP:
    out = relu(relu(U@W1 + d*b1)@W2 + d*b2) / d,   U = Z^T @ Wv
so the MLP runs transposed with stationary weights and d*b enters via
rank-1 matmuls (den = ones @ (acc + e_last): the last tile's e never
touches the DVE accumulator — it rides a second accumulating matmul).
The final layer is re-flipped: per 128-row block, lhsT = hidden block
(SBUF) x W2 puts the output non-transposed in PSUM, and relu with the
per-row 1/d scale (ACT/DVE alternating) writes the staging tile
directly — no transposes and no extra PSUM->SBUF copy in the tail.
"""
import numpy as np
import ml_dtypes
import concourse.bass as bass
from concourse import bacc
import concourse.mybir as mybir
from concourse.tile import TileContext
from concourse.bass_utils import run_bass_kernel_spmd

N = 8192
D = 128
NC = 8
RPC = N // NC          # rows per core = 1024
JT = N // 128          # j tiles = 64
F32 = mybir.dt.float32
F32R = mybir.dt.float32r
BF16 = mybir.dt.bfloat16
FP8 = mybir.dt.float8e4
MASK_D = 240.0         # fp8e4 max finite
STAB = 30.0            # global score shift, cancels in softmax
HT_CHUNKS = 4
GRP = 8                # j-tiles per adj DMA batch group
N_WARM = 26            # HAM-warmup matmuls at kernel start

PE_MASKED = ([5, 7] + [8 + r for r in (1, 3, 5, 7)]
             + [8 * g + r for g in range(2, 8) for r in (1, 4, 7)])
DVE_MASKED = [jt for jt in range(JT) if jt not in PE_MASKED]
NE = len(PE_MASKED)    # 24
NO = len(DVE_MASKED)   # 40
E_IDX = {jt: i for i, jt in enumerate(PE_MASKED)}
O_IDX = {jt: i for i, jt in enumerate(DVE_MASKED)}
E_CNT = [sum(1 for jt in PE_MASKED if jt // 8 == g) for g in range(8)]
O_CNT = [sum(1 for jt in DVE_MASKED if jt // 8 == g) for g in range(8)]
E_BASE = [sum(E_CNT[:g]) for g in range(8)]
O_BASE = [sum(O_CNT[:g]) for g in range(8)]

_CACHED = {}


def build(with_bias=False):
    nc = bacc.Bacc("TRN2", target_bir_lowering=False, debug=True)

    HTC = [nc.dram_tensor(f"HT{t}", [D, N // HT_CHUNKS], F32R, kind="ExternalInput")
           for t in range(HT_CHUNKS)]
    HNC = [nc.dram_tensor(f"HN{t}", [D, N // 4], BF16, kind="ExternalInput")
           for t in range(4)]  # pretiled [p, t*128+c], bf16
    QK = nc.dram_tensor("QK", [D, RPC], F32R, kind="ExternalInput")
    ADJE = nc.dram_tensor("ADJE", [NE * 128, RPC], FP8, kind="ExternalInput")
    ADJO = nc.dram_tensor("ADJO", [NO * 128, RPC], BF16, kind="ExternalInput")
    W1V = nc.dram_tensor("W1V", [D, D], F32R, kind="ExternalInput")
    W2 = nc.dram_tensor("W2", [D, D], BF16, kind="ExternalInput")
    B1R = nc.dram_tensor("B1R", [1, D], F32R, kind="ExternalInput")
    B2R = nc.dram_tensor("B2R", [1, D], F32R, kind="ExternalInput")
    I240 = nc.dram_tensor("I240", [D, D], FP8, kind="ExternalInput")
    ONES = nc.dram_tensor("ONES", [D, D], BF16, kind="ExternalInput")
    IDENT = nc.dram_tensor("IDENT", [1, 1], F32, kind="ExternalInput")
    BIASE = nc.dram_tensor("BIASE", [D, 1], F32, kind="ExternalInput")
    BIASO = nc.dram_tensor("BIASO", [D, 1], F32, kind="ExternalInput")
    OUT = nc.dram_tensor("OUT", [RPC, D], F32, kind="ExternalOutput")

    adjE_rows = ADJE.rearrange("(t p) i -> p t i", p=128)  # [p, E-tile, i]
    adjO_rows = ADJO.rearrange("(t p) i -> p t i", p=128)  # [p, O-tile, i]

    # group g covers j-tiles [g*8, g*8+8): 3 PE-masked + 5 DVE-masked
    NG = JT // GRP

    with TileContext(nc) as tc:
        with (
            tc.tile_pool(name="pers", bufs=1) as pers,
            tc.tile_pool(name="adjE", bufs=4) as adjEp,
            tc.tile_pool(name="adjO", bufs=4) as adjOp,
            tc.tile_pool(name="ep", bufs=8) as ep,
            tc.tile_pool(name="psA", bufs=3, space="PSUM") as psA,   # 3x[128,1024]
            tc.tile_pool(name="psZ", bufs=1, space="PSUM") as psZ,   # Z accumulator
        ):
            # ---- persistent tiles
            htc = []
            for t in range(HT_CHUNKS):
                htc_t = pers.tile([D, N // HT_CHUNKS], F32R, tag=f"ht{t}")
                htc.append(htc_t)
            hnc = []
            for t in range(4):
                hnc_t = pers.tile([D, N // 4], BF16, tag=f"hn{t}")
                hnc.append(hnc_t)
            qk = pers.tile([D, RPC], F32R, tag="qk")
            w1v = pers.tile([D, D], F32R, tag="w1v")
            w2 = pers.tile([D, D], BF16, tag="w2")
            b1r = pers.tile([1, D], F32R, tag="b1r")
            b2r = pers.tile([1, D], F32R, tag="b2r")
            i240 = pers.tile([D, D], FP8, tag="i240")
            ones = pers.tile([D, D], BF16, tag="ones")
            ident = pers.tile([1, 1], F32, tag="ident")
            biase = pers.tile([D, 1], F32, tag="biase")
            biaso = pers.tile([D, 1], F32, tag="biaso")

            def e_group_tile(g):
                return adjEp.tile([128, 4 * RPC], FP8, tag="ae", name=f"ae{g}")

            def o_group_tile(g):
                return adjOp.tile([128, 6 * RPC], BF16, tag="ao", name=f"ao{g}")

            def e_base(g):  # first ADJE tile index of group g
                return E_BASE[g]

            def o_base(g):
                return O_BASE[g]

            # critical-path DMAs on the sync queue, most-urgent first
            # (packets drain roughly in issue order); bulk/late tensors go
            # on gpsimd's software queue.
            aE0 = e_group_tile(0)
            aO0 = o_group_tile(0)
            aE1 = e_group_tile(1)
            aO1 = o_group_tile(1)
            aE = {0: aE0, 1: aE1}
            aO = {0: aO0, 1: aO1}

            hchalf0 = N // HT_CHUNKS // 2

            def adj_tile_dma(jt):
                """Issue the DMA for j-tile jt's adj slice (per-tile grain)."""
                g = jt // GRP
                if jt in E_IDX:
                    half = E_IDX[jt] - e_base(g)
                    nc.sync.dma_start(out=aE[g][:, half * RPC:(half + 1) * RPC],
                                      in_=adjE_rows[:, E_IDX[jt]])
                else:
                    half = O_IDX[jt] - o_base(g)
                    nc.sync.dma_start(out=aO[g][:, half * RPC:(half + 1) * RPC],
                                      in_=adjO_rows[:, O_IDX[jt]])

            nc.sync.dma_start(out=qk[:, 0:512], in_=QK[:, 0:512])
            nc.sync.dma_start(out=biaso[:], in_=BIASO[:])
            nc.sync.dma_start(out=htc[0][:, 0:256], in_=HTC[0][:, 0:256])
            nc.sync.dma_start(out=qk[:, 512:1024], in_=QK[:, 512:1024])
            adj_tile_dma(0)
            nc.sync.dma_start(out=htc[0][:, 256:768], in_=HTC[0][:, 256:768])
            # first Z weights on the fast queue: hnc0's head columns feed
            # Z(0..3); the slow gpsimd software queue ships the rest.
            nc.sync.dma_start(out=hnc[0][:, 0:512], in_=HNC[0][:, 0:512])
            nc.sync.dma_start(out=i240[:], in_=I240[:])
            adj_tile_dma(1)
            nc.sync.dma_start(out=aE0[:, 0:E_CNT[0] * RPC].rearrange(
                "p (k i) -> p k i", k=E_CNT[0]),
                in_=adjE_rows[:, 0:E_CNT[0]])
            nc.sync.dma_start(out=biase[:], in_=BIASE[:])
            adj_tile_dma(2)
            nc.sync.dma_start(out=htc[0][:, 768:N // HT_CHUNKS],
                              in_=HTC[0][:, 768:N // HT_CHUNKS])
            adj_tile_dma(3)
            adj_tile_dma(4)
            adj_tile_dma(6)
            for gg in (1, 2):
                if gg == 2:
                    aE[2] = e_group_tile(2)
                    aO[2] = o_group_tile(2)
                    nc.sync.dma_start(out=htc[1][:, 0:hchalf0],
                                      in_=HTC[1][:, 0:hchalf0])
                nc.sync.dma_start(
                    out=aE[gg][:, 0:E_CNT[gg] * RPC].rearrange(
                        "p (k i) -> p k i", k=E_CNT[gg]),
                    in_=adjE_rows[:, E_BASE[gg]:E_BASE[gg] + E_CNT[gg]])
                nc.sync.dma_start(
                    out=aO[gg][:, 0:O_CNT[gg] * RPC].rearrange(
                        "p (k i) -> p k i", k=O_CNT[gg]),
                    in_=adjO_rows[:, O_BASE[gg]:O_BASE[gg] + O_CNT[gg]])
                if gg == 1:
                    nc.sync.dma_start(out=hnc[0][:, 512:], in_=HNC[0][:, 512:])
                else:
                    nc.sync.dma_start(out=htc[1][:, hchalf0:],
                                      in_=HTC[1][:, hchalf0:])
            for t, src in [(ident, IDENT), (w1v, W1V), (w2, W2),
                           (b1r, B1R), (b2r, B2R), (ones, ONES)]:
                nc.gpsimd.dma_start(out=t[:], in_=src[:])
            nc.gpsimd.dma_start(out=hnc[1][:], in_=HNC[1][:])

            acc = pers.tile([D, RPC], BF16, tag="acc")
            zsb = pers.tile([D, RPC], F32R, tag="zsb")
            hts = pers.tile([D, RPC], BF16, tag="hts")    # hidden^T (SBUF, bf16)
            dentr = pers.tile([1, RPC], F32R, tag="dentr")
            rcol = pers.tile([D, NC], F32, tag="rcol")
            outsb = pers.tile([D, NC * D], F32, tag="outsb")

            # ---- stage 1 (Z matmuls lag two j-tiles so scores(jt+1)
            # issue while exp(jt) runs)
            zps = psZ.tile([D, RPC], F32, tag="z")

            # HAM warmup: the PE sits DMA-gated ~13us at kernel start and the
            # clock gate (K=4/8, 1.2GHz) only releases after ~3.4us of
            # sustained matmul activity. Without this the first ~80 real
            # matmuls run at half clock (~18us lost). Run dependency-free
            # matmuls on memset zeros into the psZ banks (the first real Z
            # matmul has start=True, so the garbage is cleared) while DMAs
            # stream in. The dummy exp pulls ACT_TABLE_LOAD off the critical
            # path too.
            warm = pers.tile([D, 512], BF16, tag="warm")
            nc.vector.memset(warm[:], 0.0)
            nc.scalar.activation(warm[:, 0:1], warm[:, 0:1],
                                 mybir.ActivationFunctionType.Exp)
            for w in range(N_WARM):
                nc.tensor.matmul(zps[:, 0:512], lhsT=warm[:, 0:128],
                                 rhs=warm[:], start=True, stop=True)

            etiles = {}

            def do_z(jt):
                e_prev = etiles.pop(jt)
                htile = hnc[jt // 16][:, (jt % 16) * 128:(jt % 16 + 1) * 128]
                for h in range(2):
                    cs = slice(h * 512, (h + 1) * 512)
                    nc.tensor.matmul(zps[:, cs], lhsT=htile, rhs=e_prev[:, cs],
                                     start=(jt == 0), stop=(jt == JT - 1))

            cwq = N // HT_CHUNKS // 128

            for jt in range(JT):
                g, r = divmod(jt, GRP)
                # per-tile prefetch 2 groups ahead keeps the wires smooth:
                # one adj tile per iteration, H chunks in halves mid-group.
                tgt = jt + 3 * GRP
                if tgt < JT:
                    if tgt % GRP == 0:
                        gg = tgt // GRP
                        aE[gg] = e_group_tile(gg)
                        aO[gg] = o_group_tile(gg)
                    adj_tile_dma(tgt)
                hchalf = N // HT_CHUNKS // 2
                for gc in (2, 3):
                    if jt == 16 * gc - 14:
                        nc.sync.dma_start(out=htc[gc][:, 0:hchalf],
                                          in_=HTC[gc][:, 0:hchalf])
                    elif jt == 16 * gc - 10:
                        nc.sync.dma_start(out=htc[gc][:, hchalf:],
                                          in_=HTC[gc][:, hchalf:])
                for gc in (2, 3):
                    if jt == 16 * gc - 6:
                        nc.sync.dma_start(out=hnc[gc][:], in_=HNC[gc][:])
                even = jt in E_IDX
                sps = psA.tile([D, RPC], F32, tag="big")
                ktile = htc[jt // cwq][:, (jt % cwq) * 128:(jt % cwq + 1) * 128]
                # Both mask halves BEFORE both score halves: an accumulating
                # matmul (start=False) stalls ~420ns until its same-bank
                # predecessor drains; grouping [m0,m1,s0,s1] makes each
                # pair's gap >= 432ns of other matmuls, hiding the drain.
                if even:
                    half = E_IDX[jt] - e_base(g)
                    adj_sb = aE[g]
                    for h in range(2):
                        cs = slice(h * 512, (h + 1) * 512)
                        nc.tensor.matmul(sps[:, cs], lhsT=i240[:],
                                         rhs=adj_sb[:, half * RPC + h * 512:
                                                    half * RPC + (h + 1) * 512],
                                         start=True, stop=False)
                for h in range(2):
                    cs = slice(h * 512, (h + 1) * 512)
                    nc.tensor.matmul(sps[:, cs], lhsT=ktile, rhs=qk[:, cs],
                                     start=not even, stop=True)
                e = ep.tile([D, RPC], BF16, tag="e")
                nc.scalar.activation(e[:], sps[:],
                                     mybir.ActivationFunctionType.Exp,
                                     bias=(biase[:] if even else biaso[:]))
                etiles[jt] = e
                if not even:
                    half = O_IDX[jt] - o_base(g)
                    adjo_sb = aO[g]
                    nc.vector.tensor_mul(e[:], e[:],
                                         adjo_sb[:, half * RPC:(half + 1) * RPC])
                if jt == 0:
                    nc.vector.tensor_copy(acc[:], e[:])
                elif jt < JT - 1:
                    nc.vector.tensor_add(acc[:], acc[:], e[:])
                if jt >= 2:
                    do_z(jt - 2)
            do_z(JT - 2)
            e_last = etiles[JT - 1]

            # ---- stage 2: denominators + normalization-deferred transposed MLP
            # z63 first: it gates the zsb -> gps -> hts -> blocks chain,
            # while the den matmuls only feed the (slack-rich) rcol path.
            do_z(JT - 1)
            # den = ones @ (acc + e_last): e(63)'s DVE add is skipped — it
            # rides a second accumulating matmul instead.
            dps = psA.tile([D, RPC], F32, tag="big")
            for h in range(2):
                cs = slice(h * 512, (h + 1) * 512)
                nc.tensor.matmul(dps[:, cs], lhsT=ones[:], rhs=acc[:, cs],
                                 start=True, stop=False)
            for h in range(2):
                cs = slice(h * 512, (h + 1) * 512)
                nc.tensor.matmul(dps[:, cs], lhsT=ones[:], rhs=e_last[:, cs],
                                 start=False, stop=True)
            nc.vector.tensor_copy(dentr[:, 0:512], dps[0:1, 0:512])
            nc.scalar.copy(zsb[:, 0:512], zps[:, 0:512])
            nc.vector.tensor_copy(dentr[:, 512:1024], dps[0:1, 512:1024])
            nc.vector.tensor_copy(zsb[:, 512:1024], zps[:, 512:1024])
            gps = psA.tile([D, RPC], F32, tag="big")
            for h in range(2):
                cs = slice(h * 512, (h + 1) * 512)
                nc.tensor.matmul(gps[:, cs], lhsT=w1v[:], rhs=zsb[:, cs],
                                 start=True, stop=not with_bias)
                if with_bias:
                    nc.tensor.matmul(gps[:, cs], lhsT=b1r[:], rhs=dentr[:, cs],
                                     start=False, stop=True)
            # 1/denom columns via tiny PE transposes of the den row
            rps = psA.tile([D, RPC], F32, tag="big")
            for it in range(NC):
                nc.tensor.transpose(rps[:, it:it + 1],
                                    dentr[0:1, it * 128:(it + 1) * 128].bitcast(F32),
                                    ident[0:1, 0:1])
            nc.scalar.activation(hts[:, 0:512], gps[:, 0:512],
                                 mybir.ActivationFunctionType.Relu)
            nc.vector.tensor_relu(hts[:, 512:1024], gps[:, 512:1024])
            nc.vector.reciprocal(rcol[:], rps[:, 0:NC])
            # final layer, re-flipped per 128-row block: lhsT = hts block so
            # the output block lands non-transposed in PSUM; relu + (1/d)
            # scale alternates ACT/DVE and writes the staging tile directly.
            outv = OUT.rearrange("(t p) d -> p t d", p=128)
            for it in range(NC):
                bps = psA.tile([D, RPC], F32, tag="big")
                nc.tensor.matmul(bps[:, 0:D], lhsT=hts[:, it * 128:(it + 1) * 128],
                                 rhs=w2[:], start=True, stop=not with_bias)
                if with_bias:
                    nc.tensor.matmul(bps[:, 0:D],
                                     lhsT=dentr[0:1, it * 128:(it + 1) * 128],
                                     rhs=b2r[:], start=False, stop=True)
                ob = outsb[:, it * 128:(it + 1) * 128]
                if it % 2 == 0:
                    nc.scalar.activation(ob, bps[:, 0:D],
                                         mybir.ActivationFunctionType.Relu,
                                         scale=rcol[:, it:it + 1])
                else:
                    nc.vector.tensor_scalar(ob, bps[:, 0:D], rcol[:, it:it + 1],
                                            0.0, op0=mybir.AluOpType.mult,
                                            op1=mybir.AluOpType.max)
                if it % 2 == 1:
                    nc.sync.dma_start(
                        out=outv[:, it - 1:it + 1],
                        in_=outsb[:, (it - 1) * D:(it + 1) * D].rearrange(
                            "p (t d) -> p t d", t=2))
    nc.finalize()
    return nc


def _prep(H, adj, Wq, Wk, Wv, W1, b1, W2, b2):
    f8 = ml_dtypes.float8_e4m3
    bf = ml_dtypes.bfloat16
    H32 = np.asarray(H, dtype=np.float32)
    HT = np.ascontiguousarray(H32.T)
    adj = np.asarray(adj)
    M = (np.asarray(Wq, np.float32) @ np.asarray(Wk, np.float32).T)
    base = {
        "W1V": np.ascontiguousarray(np.asarray(Wv, np.float32) @ np.asarray(W1, np.float32)),
        "W2": np.asarray(W2, np.float32).astype(bf),
        "B1R": np.asarray(b1, np.float32).reshape(1, D),
        "B2R": np.asarray(b2, np.float32).reshape(1, D),
        "I240": (np.eye(D, dtype=np.float32) * MASK_D).astype(f8),
        "ONES": np.ones((D, D), bf),
        "IDENT": np.eye(1, dtype=np.float32),
        "BIASE": np.full((D, 1), -(MASK_D + STAB), np.float32),
        "BIASO": np.full((D, 1), -STAB, np.float32),
    }
    cw = N // HT_CHUNKS
    for t in range(HT_CHUNKS):
        base[f"HT{t}"] = np.ascontiguousarray(HT[:, t * cw:(t + 1) * cw])
    HNP = np.ascontiguousarray(
        H32.reshape(JT, 128, D).transpose(1, 0, 2).reshape(128, N)).astype(bf)
    for t in range(4):
        base[f"HN{t}"] = np.ascontiguousarray(HNP[:, t * (N // 4):(t + 1) * (N // 4)])
    in_maps = []
    for c in range(NC):
        m = dict(base)
        m["QK"] = np.ascontiguousarray(M.T @ HT[:, c * RPC:(c + 1) * RPC])
        adjT4 = np.ascontiguousarray(
            adj[c * RPC:(c + 1) * RPC, :].T).reshape(JT, 128, RPC)
        m["ADJE"] = np.ascontiguousarray(
            adjT4[PE_MASKED].reshape(NE * 128, RPC)).astype(np.float32).astype(f8)
        m["ADJO"] = np.ascontiguousarray(
            adjT4[DVE_MASKED].reshape(NO * 128, RPC)).astype(np.float32).astype(bf)
        in_maps.append(m)
    return in_maps


def kernel(H, adj, Wq, Wk, Wv, W1, b1, W2, b2):
    wb = bool(np.any(np.asarray(b1)) or np.any(np.asarray(b2)))
    key = f"nc{int(wb)}"
    if key not in _CACHED:
        _CACHED[key] = build(with_bias=wb)
    in_maps = _prep(H, adj, Wq, Wk, Wv, W1, b1, W2, b2)
    res = run_bass_kernel_spmd(_CACHED[key], in_maps, list(range(NC)))
    return np.concatenate([res.results[c]["OUT"] for c in range(NC)], axis=0)



# revision 14
# speedup vs baseline: 1.0316x; 1.0179x over previous
"""TRN2 Bass kernel for nn_AttentionMP (GNN message passing attention).

Row-parallel attention across 8 NeuronCores: core c owns query rows
[c*1024, (c+1)*1024). Scores are computed TRANSPOSED, sT[j, i] (j = key
index on partitions, i = this core's query rows on the free dim), so
att^T feeds downstream matmuls as the moving operand with no on-device
transposes. Data-independent products fold on the host: qk =
(Wq Wk^T)^T Hq^T ships per-core; the v-projection + first MLP layer
fold into the Z weights (below).

Denominator-in-Z: W1v = Wv@W1 is SVD-truncated to rank 127 (relative
tail 2.8e-5, negligible) so the Z accumulator's 128 PSUM rows hold
[den; A^T H^T e] where A = U[:,:127]*S[:127] and row 0 comes from a
ones-column spliced into the pretiled H-chunks. This removes the
serial per-tile DVE accumulation chain for the softmax denominator
entirely (~43us of DVE), accumulates den in f32 instead of bf16, and
folds b1 in for free (gps lhsT row 0 = b1, rows 1.. = B = Vt[:127]).

Masking (uniform exp bias -30 for every tile):
 - K1 tiles (front 5 + spread): additive on the PE — madj = 240*(adj-1)
   in {-240, 0} ships as fp8 and accumulates into the scores PSUM via an
   identity-stationary matmul; exp(s-240-30) underflows to 0 in bf16.
   Both mask halves issue BEFORE both score halves: an accumulating
   matmul (start=False) stalls ~420ns until its same-bank predecessor
   drains, and the [m0,m1,s0,s1] order hides that under other matmuls.
 - remaining tiles: multiplicative on the DVE from BITPACKED adjacency:
   adj ships as uint16 (bit b of group g = tile 16g+b -> 1MB instead of
   13MB), a both-bitwise tensor_scalar extracts (P>>b)&1 to uint16
   (~420ns), and a mixed uint16*bf16 tensor_mul masks e in place
   (~690ns, still 2x_1p).

exp runs at [128,1536] granularity from a 2-deep PSUM pool (3 banks per
buffer + 2 for Z = 8) into a 12-tile bf16 e-ring; the 222-cycle ACT
overhead amortizes over 1.5 tiles. ~14 dependency-free warmup matmuls
on memset zeros run into the Z banks during the DMA-gated start so the
PE clock gate (K=4/8 until ~3.4us of sustained activity) releases
before real work arrives; the first real Z matmul's start=True clears
the garbage. A dummy exp pulls ACT_TABLE_LOAD off the critical path.

Stage 2 (normalization deferred through the MLP since relu commutes
with positive row scaling): zsb <- Z PSUM, gps = [b1;B]^T zsb gives the
hidden pre-activations scaled by den, relu -> hts (bf16), and the final
layer re-flips per 128-row block (lhsT = hts block x W2) so relu with
the per-row 1/den scale (ACT/DVE alternating) writes the output staging
tile directly. 1/den columns come from tiny PE transposes of Z row 0.
"""
import numpy as np
import ml_dtypes
import concourse.bass as bass
from concourse import bacc
import concourse.mybir as mybir
from concourse.tile import TileContext
from concourse.bass_utils import run_bass_kernel_spmd

N = 8192
D = 128
NC = 8
RPC = N // NC          # rows per core = 1024
JT = N // 128          # j tiles = 64
F32 = mybir.dt.float32
F32R = mybir.dt.float32r
BF16 = mybir.dt.bfloat16
FP8 = mybir.dt.float8e4
U16 = mybir.dt.uint16
MASK_D = 240.0         # fp8e4 max finite
STAB = 30.0            # global score shift, cancels in softmax
HT_CHUNKS = 4
N_WARM = 14            # HAM-warmup matmuls at kernel start
CHUNK = 1536           # exp granularity (cols); 3 PSUM banks
NCHUNK = (JT * RPC + CHUNK - 1) // CHUNK   # 43 (last one 1024 wide)
RING = 12 * RPC        # e-ring: 12 tiles (LCM of 1024/1536 grain)
ZLAG = 3               # Z matmuls trail the score/exp front

K1 = [0, 1, 2, 3, 4, 12, 20, 28, 36, 44, 52, 60]   # PE-masked tiles
K1_IDX = {jt: i for i, jt in enumerate(K1)}
NK1 = len(K1)

_CACHED = {}


def build(with_bias=False):
    nc = bacc.Bacc("TRN2", target_bir_lowering=False, debug=True)

    HTC = [nc.dram_tensor(f"HT{t}", [D, N // HT_CHUNKS], F32R, kind="ExternalInput")
           for t in range(HT_CHUNKS)]
    HNC = [nc.dram_tensor(f"HN{t}", [D, N // 4], BF16, kind="ExternalInput")
           for t in range(4)]  # pretiled [p, t*128+c], bf16; col c=0 is ones
    QK = nc.dram_tensor("QK", [D, RPC], F32R, kind="ExternalInput")
    MADJ = nc.dram_tensor("MADJ", [NK1 * 128, RPC], FP8, kind="ExternalInput")
    ADJP = nc.dram_tensor("ADJP", [D, 4 * RPC], U16, kind="ExternalInput")
    W1B = nc.dram_tensor("W1B", [D, D], F32R, kind="ExternalInput")
    W2 = nc.dram_tensor("W2", [D, D], BF16, kind="ExternalInput")
    B2R = nc.dram_tensor("B2R", [1, D], F32R, kind="ExternalInput")
    IDF8 = nc.dram_tensor("IDF8", [D, D], FP8, kind="ExternalInput")
    IDENT = nc.dram_tensor("IDENT", [1, 1], F32, kind="ExternalInput")
    OUT = nc.dram_tensor("OUT", [RPC, D], F32, kind="ExternalOutput")

    madj_rows = MADJ.rearrange("(t p) i -> p t i", p=128)  # [p, K1-idx, i]

    with TileContext(nc) as tc:
        with (
            tc.tile_pool(name="pers", bufs=1) as pers,
            tc.tile_pool(name="madj", bufs=3) as madjp,
            tc.tile_pool(name="msk", bufs=4) as mskp,
            tc.tile_pool(name="psA", bufs=2, space="PSUM") as psA,   # 2x[128,1536]
            tc.tile_pool(name="psZ", bufs=1, space="PSUM") as psZ,   # Z accumulator
        ):
            # ---- persistent tiles
            htc = [pers.tile([D, N // HT_CHUNKS], F32R, tag=f"ht{t}",
                             name=f"htc{t}") for t in range(HT_CHUNKS)]
            hnc = [pers.tile([D, N // 4], BF16, tag=f"hn{t}", name=f"hnc{t}")
                   for t in range(4)]
            qk = pers.tile([D, RPC], F32R, tag="qk")
            adjp = pers.tile([D, 4 * RPC], U16, tag="adjp")
            w1b = pers.tile([D, D], F32R, tag="w1b")
            w2 = pers.tile([D, D], BF16, tag="w2")
            b2r = pers.tile([1, D], F32R, tag="b2r")
            idf8 = pers.tile([D, D], FP8, tag="idf8")
            ident = pers.tile([1, 1], F32, tag="ident")
            ering = pers.tile([D, RING], BF16, tag="ering")
            zsb = pers.tile([D, RPC], F32R, tag="zsb")
            biasm = pers.tile([D, 1], F32, tag="biasm")
            hts = pers.tile([D, RPC], BF16, tag="hts")
            rcol = pers.tile([D, NC], F32, tag="rcol")
            outsb = pers.tile([D, NC * D], F32, tag="outsb")

            madj_sb = {}   # K1 tile -> (tile, ) fp8 [128, RPC]

            def madj_dma(jt, engine=None):
                t = madjp.tile([128, RPC], FP8, tag="ma", name=f"ma{jt}")
                madj_sb[jt] = t
                (engine or nc.sync).dma_start(out=t[:], in_=madj_rows[:, K1_IDX[jt]])

            # ---- critical-path DMAs, most-urgent first
            nc.sync.dma_start(out=qk[:, 0:512], in_=QK[:, 0:512])
            nc.sync.dma_start(out=htc[0][:, 0:256], in_=HTC[0][:, 0:256])
            madj_dma(0)
            nc.sync.dma_start(out=qk[:, 512:1024], in_=QK[:, 512:1024])
            madj_dma(1)
            nc.sync.dma_start(out=htc[0][:, 256:768], in_=HTC[0][:, 256:768])
            madj_dma(2)
            nc.sync.dma_start(out=hnc[0][:, 0:512], in_=HNC[0][:, 0:512])
            madj_dma(3)
            nc.sync.dma_start(out=idf8[:], in_=IDF8[:])
            madj_dma(4)
            nc.sync.dma_start(out=htc[0][:, 768:N // HT_CHUNKS],
                              in_=HTC[0][:, 768:N // HT_CHUNKS])
            # packed adjacency (1MB) + late weights on the gpsimd queue
            nc.gpsimd.dma_start(out=adjp[:, 0:2048], in_=ADJP[:, 0:2048])
            hchalf = N // HT_CHUNKS // 2
            nc.sync.dma_start(out=htc[1][:, 0:hchalf], in_=HTC[1][:, 0:hchalf])
            nc.sync.dma_start(out=hnc[0][:, 512:], in_=HNC[0][:, 512:])
            nc.sync.dma_start(out=htc[1][:, hchalf:], in_=HTC[1][:, hchalf:])
            madj_dma(12)
            nc.gpsimd.dma_start(out=adjp[:, 2048:4096], in_=ADJP[:, 2048:4096])
            for t, src in [(ident, IDENT), (w1b, W1B), (w2, W2), (b2r, B2R)]:
                nc.gpsimd.dma_start(out=t[:], in_=src[:])
            nc.gpsimd.dma_start(out=hnc[1][:], in_=HNC[1][:])

            # ---- Z accumulator + HAM warmup
            zps = psZ.tile([D, RPC], F32, tag="z")
            warm = pers.tile([D, 512], BF16, tag="warm")
            nc.vector.memset(warm[:], 0.0)
            nc.vector.memset(biasm[:], -STAB)
            nc.scalar.activation(warm[:, 0:1], warm[:, 0:1],
                                 mybir.ActivationFunctionType.Exp)
            for w in range(N_WARM):
                nc.tensor.matmul(zps[:, 0:512], lhsT=warm[:, 0:128],
                                 rhs=warm[:], start=True, stop=True)

            cwq = N // HT_CHUNKS // 128

            def do_z(t):
                htile = hnc[t // 16][:, (t % 16) * 128:(t % 16 + 1) * 128]
                base = (t % 12) * RPC
                for h in range(2):
                    nc.tensor.matmul(zps[:, h * 512:(h + 1) * 512], lhsT=htile,
                                     rhs=ering[:, base + h * 512:base + (h + 1) * 512],
                                     start=(t == 0), stop=(t == JT - 1))

            def dma_cadence(jt):
                # spread madj prefetch ~16 tiles ahead
                if (jt + 16) in K1_IDX:
                    madj_dma(jt + 16)
                for gc in (2, 3):
                    if jt == 16 * gc - 14:
                        nc.sync.dma_start(out=htc[gc][:, 0:hchalf],
                                          in_=HTC[gc][:, 0:hchalf])
                    elif jt == 16 * gc - 10:
                        nc.sync.dma_start(out=htc[gc][:, hchalf:],
                                          in_=HTC[gc][:, hchalf:])
                    elif jt == 16 * gc - 6:
                        nc.sync.dma_start(out=hnc[gc][:], in_=HNC[gc][:])

            z_next = 0
            t_proc = 0   # next tile awaiting mask processing
            for c in range(NCHUNK):
                start = c * CHUNK
                end = min(start + CHUNK, JT * RPC)
                width = end - start
                slices = [(col // RPC, (col % RPC) // 512)
                          for col in range(start, end, 512)]
                for jt, h in slices:
                    if h == 0:
                        dma_cadence(jt)
                sps = psA.tile([D, CHUNK], F32, tag="big")

                def score_mm(jt, h, masked):
                    col = jt * RPC + h * 512 - start
                    ktile = htc[jt // cwq][:, (jt % cwq) * 128:(jt % cwq + 1) * 128]
                    nc.tensor.matmul(sps[:, col:col + 512], lhsT=ktile,
                                     rhs=qk[:, h * 512:(h + 1) * 512],
                                     start=not masked, stop=True)

                # mask matmuls first, then unmasked scores, masked scores last
                for jt, h in slices:
                    if jt in K1_IDX:
                        col = jt * RPC + h * 512 - start
                        nc.tensor.matmul(sps[:, col:col + 512], lhsT=idf8[:],
                                         rhs=madj_sb[jt][:, h * 512:(h + 1) * 512],
                                         start=True, stop=False)
                for jt, h in slices:
                    if jt not in K1_IDX:
                        score_mm(jt, h, False)
                for jt, h in slices:
                    if jt in K1_IDX:
                        score_mm(jt, h, True)

                rp = start % RING
                nc.scalar.activation(ering[:, rp:rp + width], sps[:, 0:width],
                                     mybir.ActivationFunctionType.Exp,
                                     bias=biasm[:])

                # multiplicative masks for tiles fully covered by exps so far
                while (t_proc + 1) * RPC <= end:
                    t = t_proc
                    t_proc += 1
                    if t not in K1_IDX:
                        g, b = t // 16, t % 16
                        m = mskp.tile([D, RPC], U16, tag="m")
                        nc.vector.tensor_scalar(
                            m[:], adjp[:, g * RPC:(g + 1) * RPC],
                            float(b), 1.0,
                            op0=mybir.AluOpType.logical_shift_right,
                            op1=mybir.AluOpType.bitwise_and)
                        base = (t % 12) * RPC
                        nc.vector.tensor_mul(ering[:, base:base + RPC],
                                             ering[:, base:base + RPC], m[:])
                # Z trails the front by ZLAG tiles
                while z_next <= t_proc - 1 - ZLAG:
                    do_z(z_next)
                    z_next += 1
            while z_next < JT:
                do_z(z_next)
                z_next += 1

            # ---- stage 2: normalization-deferred transposed MLP.
            # Z row 0 is the softmax denominator (ones column of HN).
            nc.scalar.copy(zsb[:, 0:512], zps[:, 0:512])
            nc.vector.tensor_copy(zsb[:, 512:1024], zps[:, 512:1024])
            gps = psA.tile([D, CHUNK], F32, tag="big")
            for h in range(2):
                cs = slice(h * 512, (h + 1) * 512)
                nc.tensor.matmul(gps[:, cs], lhsT=w1b[:], rhs=zsb[:, cs],
                                 start=True, stop=True)
            # 1/denom columns via tiny PE transposes of the den row
            rps = psA.tile([D, CHUNK], F32, tag="big")
            for it in range(NC):
                nc.tensor.transpose(rps[:, it:it + 1],
                                    zsb[0:1, it * 128:(it + 1) * 128].bitcast(F32),
                                    ident[0:1, 0:1])
            nc.scalar.activation(hts[:, 0:512], gps[:, 0:512],
                                 mybir.ActivationFunctionType.Relu)
            nc.vector.tensor_relu(hts[:, 512:1024], gps[:, 512:1024])
            nc.vector.reciprocal(rcol[:], rps[:, 0:NC])
            # final layer, re-flipped per 128-row block: lhsT = hts block so
            # the output block lands non-transposed in PSUM; relu + (1/d)
            # scale alternates ACT/DVE and writes the staging tile directly.
            outv = OUT.rearrange("(t p) d -> p t d", p=128)
            for it in range(NC):
                bps = psA.tile([D, CHUNK], F32, tag="big")
                nc.tensor.matmul(bps[:, 0:D], lhsT=hts[:, it * 128:(it + 1) * 128],
                                 rhs=w2[:], start=True, stop=not with_bias)
                if with_bias:
                    nc.tensor.matmul(bps[:, 0:D],
                                     lhsT=zsb[0:1, it * 128:(it + 1) * 128],
                                     rhs=b2r[:], start=False, stop=True)
                ob = outsb[:, it * 128:(it + 1) * 128]
                if it % 2 == 0:
                    nc.scalar.activation(ob, bps[:, 0:D],
                                         mybir.ActivationFunctionType.Relu,
                                         scale=rcol[:, it:it + 1])
                else:
                    nc.vector.tensor_scalar(ob, bps[:, 0:D], rcol[:, it:it + 1],
                                            0.0, op0=mybir.AluOpType.mult,
                                            op1=mybir.AluOpType.max)
                if it % 2 == 1:
                    nc.sync.dma_start(
                        out=outv[:, it - 1:it + 1],
                        in_=outsb[:, (it - 1) * D:(it + 1) * D].rearrange(
                            "p (t d) -> p t d", t=2))
    nc.finalize()
    return nc


def _prep(H, adj, Wq, Wk, Wv, W1, b1, W2, b2):
    f8 = ml_dtypes.float8_e4m3
    bf = ml_dtypes.bfloat16
    H32 = np.asarray(H, dtype=np.float32)
    HT = np.ascontiguousarray(H32.T)
    adj = np.asarray(adj)
    M = (np.asarray(Wq, np.float32) @ np.asarray(Wk, np.float32).T)
    # SVD-truncate W1v = Wv@W1 to rank 127; den rides Z row 0 (ones col).
    W1v = np.asarray(Wv, np.float32) @ np.asarray(W1, np.float32)
    U, S, Vt = np.linalg.svd(W1v.astype(np.float64))
    A = (U[:, :127] * S[:127]).astype(np.float32)      # [128,127]
    B = Vt[:127].astype(np.float32)                    # [127,128]
    w1b = np.vstack([np.asarray(b1, np.float32).reshape(1, D), B])
    HA = np.concatenate([np.ones((N, 1), np.float32), H32 @ A], axis=1)
    base = {
        "W1B": np.ascontiguousarray(w1b),
        "W2": np.asarray(W2, np.float32).astype(bf),
        "B2R": np.asarray(b2, np.float32).reshape(1, D),
        "IDF8": np.eye(D, dtype=np.float32).astype(f8),
        "IDENT": np.eye(1, dtype=np.float32),
    }
    cw = N // HT_CHUNKS
    for t in range(HT_CHUNKS):
        base[f"HT{t}"] = np.ascontiguousarray(HT[:, t * cw:(t + 1) * cw])
    HNP = np.ascontiguousarray(
        HA.reshape(JT, 128, D).transpose(1, 0, 2).reshape(128, N)).astype(bf)
    for t in range(4):
        base[f"HN{t}"] = np.ascontiguousarray(HNP[:, t * (N // 4):(t + 1) * (N // 4)])
    in_maps = []
    for c in range(NC):
        m = dict(base)
        m["QK"] = np.ascontiguousarray(M.T @ HT[:, c * RPC:(c + 1) * RPC])
        adjT4 = np.ascontiguousarray(
            adj[c * RPC:(c + 1) * RPC, :].T).reshape(JT, 128, RPC)
        m["MADJ"] = np.ascontiguousarray(
            (adjT4[K1].astype(np.float32) - 1.0) * MASK_D
        ).reshape(NK1 * 128, RPC).astype(f8)
        packed = np.zeros((4, 128, RPC), np.uint16)
        for g in range(4):
            for b in range(16):
                packed[g] |= (adjT4[g * 16 + b] > 0).astype(np.uint16) << b
        m["ADJP"] = np.ascontiguousarray(
            packed.transpose(1, 0, 2).reshape(128, 4 * RPC))
        in_maps.append(m)
    return in_maps


def kernel(H, adj, Wq, Wk, Wv, W1, b1, W2, b2):
    wb = bool(np.any(np.asarray(b2)))
    key = f"nc{int(wb)}"
    if key not in _CACHED:
        _CACHED[key] = build(with_bias=wb)
    in_maps = _prep(H, adj, Wq, Wk, Wv, W1, b1, W2, b2)
    res = run_bass_kernel_spmd(_CACHED[key], in_maps, list(range(NC)))
    return np.concatenate([res.results[c]["OUT"] for c in range(NC)], axis=0)


# revision 15
# speedup vs baseline: 1.1184x; 1.0842x over previous
"""TRN2 Bass kernel for nn_AttentionMP (GNN message passing attention).

Row-parallel attention across 8 NeuronCores: core c owns query rows
[c*1024, (c+1)*1024). Scores are computed TRANSPOSED, sT[j, i] (j = key
index on partitions, i = this core's query rows on the free dim), so
att^T feeds downstream matmuls as the moving operand with no on-device
transposes. Data-independent products fold on the host: qk =
(Wq Wk^T)^T Hq^T ships per-core; the v-projection + first MLP layer
fold into the Z weights (below).

Denominator-in-Z: W1v = Wv@W1 is SVD-truncated to rank 127 (relative
tail 2.8e-5, negligible) so the Z accumulator's 128 PSUM rows hold
[den; A^T H^T e] where A = U[:,:127]*S[:127] and row 0 comes from a
ones-column spliced into the pretiled H-chunks. This removes the
serial per-tile DVE accumulation chain for the softmax denominator
entirely (~43us of DVE), accumulates den in f32 instead of bf16, and
folds b1 in for free (gps lhsT row 0 = b1, rows 1.. = B = Vt[:127]).

Masking (uniform exp bias -30 for every tile):
 - K1 tiles (front 5 + spread): additive on the PE — madj = 240*(adj-1)
   in {-240, 0} ships as fp8 and accumulates into the scores PSUM via an
   identity-stationary matmul; exp(s-240-30) underflows to 0 in bf16.
   Both mask halves issue BEFORE both score halves: an accumulating
   matmul (start=False) stalls ~420ns until its same-bank predecessor
   drains, and the [m0,m1,s0,s1] order hides that under other matmuls.
 - remaining tiles: multiplicative on the DVE from BITPACKED adjacency:
   adj ships as uint16 (bit b of group g = tile 16g+b -> 1MB instead of
   13MB), a both-bitwise tensor_scalar extracts (P>>b)&1 to uint16
   (~420ns), and a mixed uint16*bf16 tensor_mul masks e in place
   (~690ns, still 2x_1p).

exp runs at [128,1536] granularity from a 2-deep PSUM pool (3 banks per
buffer + 2 for Z = 8) into a 12-tile bf16 e-ring; the 222-cycle ACT
overhead amortizes over 1.5 tiles. ~14 dependency-free warmup matmuls
on memset zeros run into the Z banks during the DMA-gated start so the
PE clock gate (K=4/8 until ~3.4us of sustained activity) releases
before real work arrives; the first real Z matmul's start=True clears
the garbage. A dummy exp pulls ACT_TABLE_LOAD off the critical path.

Stage 2 (normalization deferred through the MLP since relu commutes
with positive row scaling): zsb <- Z PSUM, gps = [b1;B]^T zsb gives the
hidden pre-activations scaled by den, relu -> hts (bf16), and the final
layer re-flips per 128-row block (lhsT = hts block x W2) so relu with
the per-row 1/den scale (ACT/DVE alternating) writes the output staging
tile directly. 1/den columns come from tiny PE transposes of Z row 0.
"""
import numpy as np
import ml_dtypes
import concourse.bass as bass
from concourse import bacc
import concourse.mybir as mybir
from concourse.tile import TileContext
from concourse.bass_utils import run_bass_kernel_spmd

N = 8192
D = 128
NC = 8
RPC = N // NC          # rows per core = 1024
JT = N // 128          # j tiles = 64
F32 = mybir.dt.float32
F32R = mybir.dt.float32r
BF16 = mybir.dt.bfloat16
FP8 = mybir.dt.float8e4
U16 = mybir.dt.uint16
MASK_D = 240.0         # fp8e4 max finite
STAB = 30.0            # global score shift, cancels in softmax
HT_CHUNKS = 4
N_WARM = 14            # HAM-warmup matmuls at kernel start
CHUNK = 1536           # exp granularity (cols); 3 PSUM banks
NCHUNK = (JT * RPC + CHUNK - 1) // CHUNK   # 43 (last one 1024 wide)
RING = 12 * RPC        # e-ring: 12 tiles (LCM of 1024/1536 grain)
ZLAG = 3               # Z matmuls trail the score/exp front

K1 = [6, 12, 18, 24, 30, 36, 42, 48, 54, 60]   # PE-masked tiles (spread;
# the front tiles are DVE-masked so the first exp chunks depend only on
# qk/htc DMAs, not on madj/idf8)
K1_IDX = {jt: i for i, jt in enumerate(K1)}
NK1 = len(K1)

_CACHED = {}


def build(with_bias=False):
    nc = bacc.Bacc("TRN2", target_bir_lowering=False, debug=True)

    HTC = [nc.dram_tensor(f"HT{t}", [D, N // HT_CHUNKS], F32R, kind="ExternalInput")
           for t in range(HT_CHUNKS)]
    HNC = [nc.dram_tensor(f"HN{t}", [D, N // 4], BF16, kind="ExternalInput")
           for t in range(4)]  # pretiled [p, t*128+c], bf16; col c=0 is ones
    QK = nc.dram_tensor("QK", [D, RPC], F32R, kind="ExternalInput")
    MADJ = nc.dram_tensor("MADJ", [NK1 * 128, RPC], FP8, kind="ExternalInput")
    ADJP = nc.dram_tensor("ADJP", [D, 4 * RPC], U16, kind="ExternalInput")
    W1B = nc.dram_tensor("W1B", [D, D], F32R, kind="ExternalInput")
    W2 = nc.dram_tensor("W2", [D, D], BF16, kind="ExternalInput")
    B2R = nc.dram_tensor("B2R", [1, D], F32R, kind="ExternalInput")
    IDF8 = nc.dram_tensor("IDF8", [D, D], FP8, kind="ExternalInput")
    IDENT = nc.dram_tensor("IDENT", [1, 1], F32, kind="ExternalInput")
    OUT = nc.dram_tensor("OUT", [RPC, D], F32, kind="ExternalOutput")

    madj_rows = MADJ.rearrange("(t p) i -> p t i", p=128)  # [p, K1-idx, i]

    with TileContext(nc) as tc:
        with (
            tc.tile_pool(name="pers", bufs=1) as pers,
            tc.tile_pool(name="madj", bufs=3) as madjp,
            tc.tile_pool(name="msk", bufs=4) as mskp,
            tc.tile_pool(name="psA", bufs=2, space="PSUM") as psA,   # 2x[128,1536]
            tc.tile_pool(name="psZ", bufs=1, space="PSUM") as psZ,   # Z accumulator
        ):
            # ---- persistent tiles
            htc = [pers.tile([D, N // HT_CHUNKS], F32R, tag=f"ht{t}",
                             name=f"htc{t}") for t in range(HT_CHUNKS)]
            hnc = [pers.tile([D, N // 4], BF16, tag=f"hn{t}", name=f"hnc{t}")
                   for t in range(4)]
            qk = pers.tile([D, RPC], F32R, tag="qk")
            adjp = pers.tile([D, 4 * RPC], U16, tag="adjp")
            w1b = pers.tile([D, D], F32R, tag="w1b")
            w2 = pers.tile([D, D], BF16, tag="w2")
            b2r = pers.tile([1, D], F32R, tag="b2r")
            idf8 = pers.tile([D, D], FP8, tag="idf8")
            ident = pers.tile([1, 1], F32, tag="ident")
            ering = pers.tile([D, RING], BF16, tag="ering")
            zsb = pers.tile([D, RPC], F32R, tag="zsb")
            biasm = pers.tile([D, 1], F32, tag="biasm")
            hts = pers.tile([D, RPC], BF16, tag="hts")
            rcol = pers.tile([D, NC], F32, tag="rcol")
            outsb = pers.tile([D, NC * D], F32, tag="outsb")

            madj_sb = {}   # K1 tile -> (tile, ) fp8 [128, RPC]

            def madj_dma(jt, engine=None):
                t = madjp.tile([128, RPC], FP8, tag="ma", name=f"ma{jt}")
                madj_sb[jt] = t
                (engine or nc.sync).dma_start(out=t[:], in_=madj_rows[:, K1_IDX[jt]])

            # ---- critical-path DMAs, most-urgent first. The first exp
            # chunk needs only qk + htc0's head; packed adjacency and the
            # identity ride the gpsimd software queue in parallel.
            nc.sync.dma_start(out=qk[:, 0:512], in_=QK[:, 0:512])
            nc.sync.dma_start(out=htc[0][:, 0:256], in_=HTC[0][:, 0:256])
            nc.sync.dma_start(out=qk[:, 512:1024], in_=QK[:, 512:1024])
            nc.sync.dma_start(out=htc[0][:, 256:768], in_=HTC[0][:, 256:768])
            nc.sync.dma_start(out=hnc[0][:, 0:512], in_=HNC[0][:, 0:512])
            nc.gpsimd.dma_start(out=adjp[:, 0:2048], in_=ADJP[:, 0:2048])
            nc.gpsimd.dma_start(out=idf8[:], in_=IDF8[:])
            nc.sync.dma_start(out=htc[0][:, 768:N // HT_CHUNKS],
                              in_=HTC[0][:, 768:N // HT_CHUNKS])
            madj_dma(6)
            nc.gpsimd.dma_start(out=adjp[:, 2048:4096], in_=ADJP[:, 2048:4096])
            hchalf = N // HT_CHUNKS // 2
            nc.sync.dma_start(out=htc[1][:, 0:hchalf], in_=HTC[1][:, 0:hchalf])
            nc.sync.dma_start(out=hnc[0][:, 512:], in_=HNC[0][:, 512:])
            madj_dma(12)
            nc.sync.dma_start(out=htc[1][:, hchalf:], in_=HTC[1][:, hchalf:])
            for t, src in [(ident, IDENT), (w1b, W1B), (w2, W2), (b2r, B2R)]:
                nc.gpsimd.dma_start(out=t[:], in_=src[:])
            nc.gpsimd.dma_start(out=hnc[1][:], in_=HNC[1][:])

            # ---- Z accumulator + HAM warmup
            zps = psZ.tile([D, RPC], F32, tag="z")
            warm = pers.tile([D, 512], BF16, tag="warm")
            nc.vector.memset(warm[:], 0.0)
            nc.vector.memset(biasm[:], -STAB)
            nc.scalar.activation(warm[:, 0:1], warm[:, 0:1],
                                 mybir.ActivationFunctionType.Exp)
            for w in range(N_WARM):
                nc.tensor.matmul(zps[:, 0:512], lhsT=warm[:, 0:128],
                                 rhs=warm[:], start=True, stop=True)

            cwq = N // HT_CHUNKS // 128

            def do_z(t):
                htile = hnc[t // 16][:, (t % 16) * 128:(t % 16 + 1) * 128]
                base = (t % 12) * RPC
                for h in range(2):
                    nc.tensor.matmul(zps[:, h * 512:(h + 1) * 512], lhsT=htile,
                                     rhs=ering[:, base + h * 512:base + (h + 1) * 512],
                                     start=(t == 0), stop=(t == JT - 1))

            def dma_cadence(jt):
                # spread madj prefetch ~16 tiles ahead
                if (jt + 16) in K1_IDX:
                    madj_dma(jt + 16)
                for gc in (2, 3):
                    if jt == 16 * gc - 14:
                        nc.sync.dma_start(out=htc[gc][:, 0:hchalf],
                                          in_=HTC[gc][:, 0:hchalf])
                    elif jt == 16 * gc - 10:
                        nc.sync.dma_start(out=htc[gc][:, hchalf:],
                                          in_=HTC[gc][:, hchalf:])
                    elif jt == 16 * gc - 6:
                        nc.sync.dma_start(out=hnc[gc][:], in_=HNC[gc][:])

            z_next = 0
            t_proc = 0   # next tile awaiting mask processing
            for c in range(NCHUNK):
                start = c * CHUNK
                end = min(start + CHUNK, JT * RPC)
                width = end - start
                slices = [(col // RPC, (col % RPC) // 512)
                          for col in range(start, end, 512)]
                for jt, h in slices:
                    if h == 0:
                        dma_cadence(jt)
                sps = psA.tile([D, CHUNK], F32, tag="big")

                def score_mm(jt, h, masked):
                    col = jt * RPC + h * 512 - start
                    ktile = htc[jt // cwq][:, (jt % cwq) * 128:(jt % cwq + 1) * 128]
                    nc.tensor.matmul(sps[:, col:col + 512], lhsT=ktile,
                                     rhs=qk[:, h * 512:(h + 1) * 512],
                                     start=not masked, stop=True)

                # mask matmuls first, then unmasked scores, masked scores last
                for jt, h in slices:
                    if jt in K1_IDX:
                        col = jt * RPC + h * 512 - start
                        nc.tensor.matmul(sps[:, col:col + 512], lhsT=idf8[:],
                                         rhs=madj_sb[jt][:, h * 512:(h + 1) * 512],
                                         start=True, stop=False)
                for jt, h in slices:
                    if jt not in K1_IDX:
                        score_mm(jt, h, False)
                for jt, h in slices:
                    if jt in K1_IDX:
                        score_mm(jt, h, True)

                rp = start % RING
                nc.scalar.activation(ering[:, rp:rp + width], sps[:, 0:width],
                                     mybir.ActivationFunctionType.Exp,
                                     bias=biasm[:])

                # multiplicative masks for tiles fully covered by exps so far
                while (t_proc + 1) * RPC <= end:
                    t = t_proc
                    t_proc += 1
                    if t not in K1_IDX:
                        g, b = t // 16, t % 16
                        m = mskp.tile([D, RPC], U16, tag="m")
                        nc.vector.tensor_scalar(
                            m[:], adjp[:, g * RPC:(g + 1) * RPC],
                            float(b), 1.0,
                            op0=mybir.AluOpType.logical_shift_right,
                            op1=mybir.AluOpType.bitwise_and)
                        base = (t % 12) * RPC
                        nc.vector.tensor_mul(ering[:, base:base + RPC],
                                             ering[:, base:base + RPC], m[:])
                # Z trails the front by ZLAG tiles
                while z_next <= t_proc - 1 - ZLAG:
                    do_z(z_next)
                    z_next += 1
            while z_next < JT:
                do_z(z_next)
                z_next += 1

            # ---- stage 2: normalization-deferred transposed MLP.
            # Z row 0 is the softmax denominator (ones column of HN).
            nc.scalar.copy(zsb[:, 0:512], zps[:, 0:512])
            nc.vector.tensor_copy(zsb[:, 512:1024], zps[:, 512:1024])
            gps = psA.tile([D, CHUNK], F32, tag="big")
            for h in range(2):
                cs = slice(h * 512, (h + 1) * 512)
                nc.tensor.matmul(gps[:, cs], lhsT=w1b[:], rhs=zsb[:, cs],
                                 start=True, stop=True)
            # 1/denom columns via tiny PE transposes of the den row
            rps = psA.tile([D, CHUNK], F32, tag="big")
            for it in range(NC):
                nc.tensor.transpose(rps[:, it:it + 1],
                                    zsb[0:1, it * 128:(it + 1) * 128].bitcast(F32),
                                    ident[0:1, 0:1])
            nc.scalar.activation(hts[:, 0:512], gps[:, 0:512],
                                 mybir.ActivationFunctionType.Relu)
            nc.vector.tensor_relu(hts[:, 512:1024], gps[:, 512:1024])
            nc.vector.reciprocal(rcol[:], rps[:, 0:NC])
            # final layer, re-flipped per 128-row block: lhsT = hts block so
            # the output block lands non-transposed in PSUM; relu + (1/d)
            # scale alternates ACT/DVE and writes the staging tile directly.
            outv = OUT.rearrange("(t p) d -> p t d", p=128)
            for it in range(NC):
                bps = psA.tile([D, CHUNK], F32, tag="big")
                nc.tensor.matmul(bps[:, 0:D], lhsT=hts[:, it * 128:(it + 1) * 128],
                                 rhs=w2[:], start=True, stop=not with_bias)
                if with_bias:
                    nc.tensor.matmul(bps[:, 0:D],
                                     lhsT=zsb[0:1, it * 128:(it + 1) * 128],
                                     rhs=b2r[:], start=False, stop=True)
                ob = outsb[:, it * 128:(it + 1) * 128]
                if it % 2 == 0:
                    nc.scalar.activation(ob, bps[:, 0:D],
                                         mybir.ActivationFunctionType.Relu,
                                         scale=rcol[:, it:it + 1])
                else:
                    nc.vector.tensor_scalar(ob, bps[:, 0:D], rcol[:, it:it + 1],
                                            0.0, op0=mybir.AluOpType.mult,
                                            op1=mybir.AluOpType.max)
                if it % 2 == 1:
                    nc.sync.dma_start(
                        out=outv[:, it - 1:it + 1],
                        in_=outsb[:, (it - 1) * D:(it + 1) * D].rearrange(
                            "p (t d) -> p t d", t=2))
    nc.finalize()
    return nc


def _prep(H, adj, Wq, Wk, Wv, W1, b1, W2, b2):
    f8 = ml_dtypes.float8_e4m3
    bf = ml_dtypes.bfloat16
    H32 = np.asarray(H, dtype=np.float32)
    HT = np.ascontiguousarray(H32.T)
    adj = np.asarray(adj)
    M = (np.asarray(Wq, np.float32) @ np.asarray(Wk, np.float32).T)
    # SVD-truncate W1v = Wv@W1 to rank 127; den rides Z row 0 (ones col).
    W1v = np.asarray(Wv, np.float32) @ np.asarray(W1, np.float32)
    U, S, Vt = np.linalg.svd(W1v.astype(np.float64))
    A = (U[:, :127] * S[:127]).astype(np.float32)      # [128,127]
    B = Vt[:127].astype(np.float32)                    # [127,128]
    w1b = np.vstack([np.asarray(b1, np.float32).reshape(1, D), B])
    HA = np.concatenate([np.ones((N, 1), np.float32), H32 @ A], axis=1)
    base = {
        "W1B": np.ascontiguousarray(w1b),
        "W2": np.asarray(W2, np.float32).astype(bf),
        "B2R": np.asarray(b2, np.float32).reshape(1, D),
        "IDF8": np.eye(D, dtype=np.float32).astype(f8),
        "IDENT": np.eye(1, dtype=np.float32),
    }
    cw = N // HT_CHUNKS
    for t in range(HT_CHUNKS):
        base[f"HT{t}"] = np.ascontiguousarray(HT[:, t * cw:(t + 1) * cw])
    HNP = np.ascontiguousarray(
        HA.reshape(JT, 128, D).transpose(1, 0, 2).reshape(128, N)).astype(bf)
    for t in range(4):
        base[f"HN{t}"] = np.ascontiguousarray(HNP[:, t * (N // 4):(t + 1) * (N // 4)])
    in_maps = []
    for c in range(NC):
        m = dict(base)
        m["QK"] = np.ascontiguousarray(M.T @ HT[:, c * RPC:(c + 1) * RPC])
        adjT4 = np.ascontiguousarray(
            adj[c * RPC:(c + 1) * RPC, :].T).reshape(JT, 128, RPC)
        m["MADJ"] = np.ascontiguousarray(
            (adjT4[K1].astype(np.float32) - 1.0) * MASK_D
        ).reshape(NK1 * 128, RPC).astype(f8)
        packed = np.zeros((4, 128, RPC), np.uint16)
        for g in range(4):
            for b in range(16):
                packed[g] |= (adjT4[g * 16 + b] > 0).astype(np.uint16) << b
        m["ADJP"] = np.ascontiguousarray(
            packed.transpose(1, 0, 2).reshape(128, 4 * RPC))
        in_maps.append(m)
    return in_maps


def kernel(H, adj, Wq, Wk, Wv, W1, b1, W2, b2):
    wb = bool(np.any(np.asarray(b2)))
    key = f"nc{int(wb)}"
    if key not in _CACHED:
        _CACHED[key] = build(with_bias=wb)
    in_maps = _prep(H, adj, Wq, Wk, Wv, W1, b1, W2, b2)
    res = run_bass_kernel_spmd(_CACHED[key], in_maps, list(range(NC)))
    return np.concatenate([res.results[c]["OUT"] for c in range(NC)], axis=0)
